# revision 7
# baseline (speedup 1.0000x reference)
"""Correlation cost-volume kernel for Trainium2 (8 NeuronCores).

out[b,d,h,w] = sum_c left[b,c,h,w] * right[b,c,h,w-shift[d]]
  left/right: [4, 64, 256, 512] f32, shift: arange(96) -> out [4, 96, 256, 512] f32

Strategy (v4 — K=128 h-pair-packed matmuls, no DRAM scratch):
  - Shard (b, h-half) across 8 cores: per-core left/right [64, 128, 512], no halo
    (shifts are along W only), no collectives.
  - Per (h-pair, w-subtile of 32): the cost volume is a 96-wide anti-band of
    the Gram matrix G[i, j] = sum_c L[c, wg+i] * R[c, wg-95+j].  The two h
    parities are packed into ONE K=128 matmul with block-diagonal weights
    lhsT = [[L_even, 0], [0, L_odd]] ([K=128, M=64]) and rhs = R_even/R_odd
    stacked on partitions 0-63/64-127 ([K=128, N=127]).  Each streamed rhs
    column uses the full 128-partition XBUS width, halving the TensorE
    streaming time vs two K=64 matmuls (the stream, not FLOPs, is the limit).
    The weights must be one contiguous free dim, so the host ships L as
    32-col interleaved blocks [A_k | B_k] (A_k = L_even on parts 0-63,
    B_k = L_odd on parts 64-127, zeros opposite): window wg = cols [2wg, +64).
  - MM(t, g) writes PSUM bank (g//2) at partitions 64*(g%2), cols t*127;
    consecutive MMs alternate PE column strips so LDWEIGHTS pulls ahead.
  - PSUM -> SBUF copies split across Vector (bank0) and Scalar (bank1); raw
    127-wide Gram rows go straight to output DRAM as one clean contiguous
    DMA per h-pair (2032-byte runs).  Input DMAs issue from the gpsimd ring.
  - The 96-diagonal band shear (j = i_l + 95 - d) is undone on the HOST with
    a zero-copy as_strided view — no diagonal DMAs, no DRAM scratch.
  - Host: pack/cast inputs to bf16, de-shear + upcast + transpose the output.
"""
import sys

sys.path.insert(0, "/opt/trn_rl_repo")

import numpy as np
import ml_dtypes

import concourse.bass as bass
import concourse.mybir as mybir
import concourse.tile as tile
from concourse.ap import AP
from concourse.bass_utils import run_bass_kernel_spmd
from concourse.vector_clock import ScopedClock

B, C, H, W, D = 4, 64, 256, 512, 96
HC = H // 2          # 128 h rows per core
T = 32               # w-subtile size
NT4 = 4              # w-chunks of 128 per h row
NG = T + D - 1       # 127 gram columns per subtile
BLK = 16             # h rows per block
NBLK = HC // BLK     # 8 blocks
# per-pair SBUF cols: [95 pad][512 R][1024 Lbd interleaved]
PAIR_COLS = (D - 1) + W + 2 * W  # 1631
R_OFF = D - 1        # R data starts at col 95
L_OFF = (D - 1) + W  # Lbd: 32-col blocks [L_even blk k | L_odd blk k] ...
ROW = 2 * NT4 * NG   # out cols per h-pair: (bank, t, j) = 2*4*127 = 1016

BF16 = mybir.dt.bfloat16
F32 = mybir.dt.float32


_orig_add_instruction = tile.TileContext._add_instruction


def _patched_add_instruction(self, inst):
    # This walrus build allows at most ONE sync-wait per instruction: peel
    # extra waits onto single-wait NOPs on the same engine, just before it.
    si = inst.sync_info
    if si is not None and len(si.on_wait) > 1:
        waits = list(si.on_wait)
        for w in waits[:-1]:
            nop = mybir.InstNoOp(
                name=self.nc.get_next_instruction_name(),
                text_hint="split_wait",
                bass_nofuse=True,
            )
            nop.engine = inst.engine
            nop.sync_info = mybir.SyncInfo(on_wait=[w], on_update=[])
            _orig_add_instruction(self, nop)
        si.on_wait = waits[-1:]
    _orig_add_instruction(self, inst)


tile.TileContext._add_instruction = _patched_add_instruction


def _patched_drain_and_barrier(self, tick_clock, wait_clock):
    # This walrus build allows only ONE sync-wait on the tail Drain CTRL
    # instruction; split the final-clock waits across single-wait NOPs.
    nc = self.nc
    probe = nc.sync.nop(nofuse=True, hint="drain_waits")
    wait_clock.add_sem_waits(probe.ins, ScopedClock({None: tick_clock.global_clock}))
    waits = list(probe.ins.sync_info.on_wait)
    probe.ins.sync_info.on_wait = waits[:1]
    for w in waits[1:]:
        n = nc.sync.nop(nofuse=True, hint="drain_waits")
        n.ins.sync_info = mybir.SyncInfo(on_wait=[w], on_update=[])
    nc.sync.drain()
    nc.all_engine_barrier()
    assert self.sems is not None
    popped = nc._tile_sem_poison_stack.pop()
    assert popped is self._sem_poison
    nc.clear_and_free_semaphores(list(self.sems.allocated().values()))
    nc.all_engine_barrier()


tile.TileContext._drain_and_barrier = _patched_drain_and_barrier


def build_graph():
    nc = bass.Bass()
    lr_ext = nc.declare_dram_parameter("lrpack", [128, HC // 2, 3 * W], BF16, isOutput=False)
    # raw (sheared) band rows: [h-pair, partition(=64*gl+32*par+i_l), (bank, t, j)]
    out_ext = nc.declare_dram_parameter("out", [HC // 2, 128, ROW], BF16, isOutput=True)

    with tile.TileContext(nc) as tc:
        IN_BUFS = 4
        with (
            tc.tile_pool(name="inp", bufs=IN_BUFS) as in_pool,
            tc.tile_pool(name="outsb", bufs=8) as out_pool,
            tc.tile_pool(name="psum", bufs=4, space="PSUM") as psum_pool,
        ):
            for blk in range(NBLK):
                # ---- load one block: 8 h-pairs -------------------------------
                blk_tile = in_pool.tile([128, (BLK // 2) * PAIR_COLS], BF16)
                pitch = blk_tile.tensor.shape[1]
                # The 95-col pad is never overwritten (input DMAs touch
                # disjoint cells), so zero each rotating slot exactly once.
                if blk < IN_BUFS:
                    pad_ap = AP(
                        tensor=blk_tile.tensor,
                        offset=blk_tile.offset,
                        ap=[[pitch, 128], [PAIR_COLS, BLK // 2], [1, R_OFF]],
                    )
                    nc.gpsimd.memset(pad_ap, 0.0)
                h2_0 = blk * (BLK // 2)
                # per-pair input DMAs on the gpsimd ring (input never queues
                # behind output DMAs, which issue from the sync ring)
                for j2 in range(BLK // 2):
                    base = blk_tile.offset + j2 * PAIR_COLS
                    dst_r = AP(
                        tensor=blk_tile.tensor,
                        offset=base + R_OFF,
                        ap=[[pitch, 128], [1, W]],
                    )
                    nc.gpsimd.dma_start(dst_r, lr_ext[:, h2_0 + j2, 0:W])
                    dst_lbd = AP(
                        tensor=blk_tile.tensor,
                        offset=base + L_OFF,
                        ap=[[pitch, 128], [1, 2 * W]],
                    )
                    nc.gpsimd.dma_start(dst_lbd, lr_ext[:, h2_0 + j2, W : 3 * W])

                # ---- compute: per h-pair, 16 K=128 matmuls -------------------
                for j2 in range(BLK // 2):
                    base = j2 * PAIR_COLS
                    ps0 = psum_pool.tile([128, NT4 * NG], F32)
                    ps1 = psum_pool.tile([128, NT4 * NG], F32)
                    out_sb = out_pool.tile([128, ROW], BF16)
                    for bank, ps in ((0, ps0), (1, ps1)):
                        for t in range(NT4):
                            for gl in range(2):
                                g = 2 * bank + gl
                                wg = t * 128 + T * g
                                # block-diagonal weights [K=128, M=64]: the
                                # host-interleaved Lbd gives window wg as one
                                # contiguous 64-col block [L_e blk | L_o blk].
                                lhsT = blk_tile[0:128, base + L_OFF + 2 * wg : base + L_OFF + 2 * wg + 64]
                                rhs = blk_tile[0:128, base + wg : base + wg + NG]
                                nc.tensor.matmul(
                                    ps[64 * gl : 64 * gl + 64, t * NG : (t + 1) * NG],
                                    lhsT=lhsT,
                                    rhs=rhs,
                                    start=True,
                                    stop=True,
                                    tile_position=(0, 64 * gl),
                                )
                        # copy as soon as this bank's 8 MMs are done
                        if bank == 0:
                            nc.vector.tensor_copy(out_sb[:, 0 : NT4 * NG], ps0[:, 0 : NT4 * NG])
                        else:
                            nc.scalar.copy(out_sb[:, NT4 * NG : ROW], ps1[:, 0 : NT4 * NG])
                    # one clean DMA per pair: contiguous 2032-byte runs
                    nc.sync.dma_start(out_ext[blk * (BLK // 2) + j2], out_sb[:])
    return nc


_CACHED = {}


def _get_graph():
    if "nc" not in _CACHED:
        _CACHED["nc"] = build_graph()
    return _CACHED["nc"]


def _pack_core(left_b, right_b, h0):
    """left_b/right_b: [C, H, W] f32 for one batch -> lrpack [128, 64, 1536] bf16.

    Cols [0, 512): R (h-parity on partition halves).  Cols [512, 1536): Lbd,
    16 interleaved 64-col groups: group k = [L_even w-block k on parts 0-63 |
    L_odd w-block k on parts 64-127], zeros opposite (block-diagonal weights).
    """
    ls = left_b[:, h0 : h0 + HC, :]
    rs = right_b[:, h0 : h0 + HC, :]
    pack = np.zeros((128, HC // 2, 3 * W), dtype=np.float32)
    pack[0:64, :, 0:W] = rs[:, 0::2, :]
    pack[64:128, :, 0:W] = rs[:, 1::2, :]
    w = np.arange(W)
    cols_e = W + 64 * (w // T) + (w % T)
    pack[0:64, :, cols_e] = ls[:, 0::2, :]
    pack[64:128, :, cols_e + T] = ls[:, 1::2, :]
    return pack.astype(ml_dtypes.bfloat16)


def _unshear_core(oc):
    """oc: [64, 128, 1016] bf16 raw band rows -> [D, HC, W] f32.

    raw[h2, p=64*gl+32*par+i_l, bank*508 + t*127 + j] = G at
    w = t*128 + (2*bank+gl)*32 + i_l, h = 2*h2 + par, d = i_l + 95 - j.
    De-shear with a strided view: V[..., i_l, ..., d] = r[..., i_l, ..., 95+i_l-d].
    """
    r7 = oc.reshape(64, 2, 2, 32, 2, 4, NG)  # [h2, gl, par, i_l, bank, t, j]
    s = r7.strides
    v = np.lib.stride_tricks.as_strided(
        r7[:, :, :, :, :, :, 95:],
        shape=(64, 2, 2, 32, 2, 4, D),
        strides=(s[0], s[1], s[2], s[3] + s[6], s[4], s[5], -s[6]),
    )
    # v dims: [h2, gl, par, i_l, bank, t, d] -> [d, (h2, par), (t, bank, gl, i_l)]
    return v.transpose(6, 0, 2, 5, 4, 1, 3).reshape(D, HC, W).astype(np.float32)


def _run(inputs, trace=False):
    left = np.asarray(inputs["left"], dtype=np.float32)
    right = np.asarray(inputs["right"], dtype=np.float32)
    shift = np.asarray(inputs["shift"])

    nc = _get_graph()
    in_maps = []
    for core in range(8):
        b, half = core // 2, core % 2
        in_maps.append({"lrpack": _pack_core(left[b], right[b], half * HC)})

    res = run_bass_kernel_spmd(nc, in_maps, core_ids=list(range(8)), trace=trace)

    out = np.empty((B, D, H, W), dtype=np.float32)
    for core in range(8):
        b, half = core // 2, core % 2
        oc = np.asarray(res.results[core]["out"])  # [64, 128, 1016] bf16
        out[b, :, half * HC : (half + 1) * HC, :] = _unshear_core(oc)

    # band covers integer shifts 0..95; remap if shift isn't exactly arange
    s = np.asarray(shift, dtype=np.float64)
    if not np.allclose(s, np.arange(D)):
        si = np.rint(s).astype(np.int64)
        if np.allclose(s, si) and si.min() >= 0 and si.max() < D:
            out = out[:, si, :, :]
        else:
            raise NotImplementedError(f"unsupported shift vector: {s}")
    return out, res


def kernel(**inputs) -> np.ndarray:
    out, _ = _run(inputs, trace=False)
    return out
